# revision 34
# baseline (speedup 1.0000x reference)
"""Trainium2 Bass kernel for MllamaTextCrossAttention (B=1, Q=1024, KV=6404,
HIDDEN=4096, 32 q-heads / 8 kv-heads, head_dim=128, fp32 in/out).

Sharding: tensor-parallel over heads across 8 cores. Core c owns kv-head c and
q-heads 4c..4c+3, plus the matching o_proj in-feature slice; each core emits a
full-shape partial output and the host sums the 8 partials.

v16: all matmul operands bf16 (host-cast).  Streaming tensors are host-blocked
into sequential slabs (~1 MB dma_starts; the HWDGE sequencer needs ~0.6us per
issue), and the cross stream alternates between both HWDGE rings (sync +
scalar) with a 10-slab prefetch that starts during phase 1.  Phases:
  1. q projection
  2. k/v projection for all kv, short tail group first so its exp-scale chain
     never straddles the phase boundary.  The q rmsnorm (broadcast-form
     sumsq + reciprocal_approx_fast) and per-group kv exp scales are
     interleaved so their serial chains hide under the projection stream.
     V is kept in [d, kv] form, then transposed chunkwise via the DMA xbar
     on the idle sync ring once the cross stream finishes
  3. attention sweep per head: scores both q-halves of one chunk into a
     1024-wide PSUM pair, one 1024-wide Exp (per-partition kscale = k-rmsnorm
     x 1/sqrt(D)), A.V accumulated across all 51 chunks in persistent PSUM;
     rowsum for q-half 0 rides the PE, q-half 1 accumulates on the
     otherwise-idle Vector engine
  4. o projection with streamed weights, output via blocked slab DMAs
Pad kv rows are killed inside the exp via a -40 per-partition bias on the
last chunk.
"""

import sys

sys.path.insert(0, "/opt/trn_rl_repo")

import numpy as np
import ml_dtypes

import concourse.bass as bass
from concourse import bacc
import concourse.mybir as mybir
import concourse.tile as tile
from concourse.bass_utils import run_bass_kernel_spmd

H = 4096          # hidden size
Q = 1024          # query length
KV = 6404         # kv length
KVP = 6528        # padded to 51 * 128
NKC = 51          # kv 128-chunks
D = 128           # head dim
HPC = 4           # q heads per core
EPS = 1e-5
F32 = mybir.dt.float32
F32R = mybir.dt.float32r
BF16 = mybir.dt.bfloat16
NPBF16 = ml_dtypes.bfloat16

KT = H // 128     # 32 contraction tiles of 128
GROUPS = [(g * 1024, min(1024, KVP - g * 1024)) for g in range(7)]
LAST_VALID = KV - 128 * (NKC - 1)   # valid kv rows in the final 128-chunk

SLAB_K = 4                          # ktiles per cross-stream slab
NSLAB = KT // SLAB_K                # 8 slabs per kv group
SLAB_K1 = 2                         # ktiles per phase-1 slab
NSLAB1 = KT // SLAB_K1
CRS_SLAB = [128 * SLAB_K * w for (_, w) in GROUPS]   # elems per slab, per g
CRS_OFF = np.cumsum([0] + [NSLAB * s for s in CRS_SLAB]).tolist()
HID_SLAB = 128 * SLAB_K1 * Q
QW_SLAB = 128 * SLAB_K1 * HPC * D
OUT_SLAB = 128 * (Q // 128) * 512


def build_nc(tc_kwargs=None):
    nc = bacc.Bacc(None)
    hid_r = nc.dram_tensor("hid_r", [H * Q], BF16, kind="ExternalInput")
    crs_r = nc.dram_tensor("crs_r", [H * KVP], BF16, kind="ExternalInput")
    qw_r = nc.dram_tensor("qw_r", [H * HPC * D], BF16, kind="ExternalInput")
    kw_r = nc.dram_tensor("kw_r", [128, KT, D], BF16, kind="ExternalInput")
    vw_r = nc.dram_tensor("vw_r", [128, KT, D], BF16, kind="ExternalInput")
    ow_r = nc.dram_tensor("ow_r", [128, HPC, H], BF16, kind="ExternalInput")
    ones_f = nc.dram_tensor("ones_f", [128, 128], F32R, kind="ExternalInput")
    ones_b = nc.dram_tensor("ones_b", [128, 128], BF16, kind="ExternalInput")
    qnw = nc.dram_tensor("qnw", [D, 1], F32, kind="ExternalInput")
    pbias_in = nc.dram_tensor("pbias", [128, 1], F32, kind="ExternalInput")
    out = nc.dram_tensor("out", [Q * H], F32, kind="ExternalOutput")

    with tile.TileContext(nc) as tc:
        with tc.tile_pool(name="const", bufs=1) as cst:
            onesf = cst.tile([128, 128], F32R)
            onesb = cst.tile([128, 128], BF16)
            qnw_t = cst.tile([D, 1], F32)
            pbias = cst.tile([128, 1], F32)
            eps_q = cst.tile([128, 1], F32)
            eps_k = cst.tile([128, 1], F32)
            scr = cst.tile([1, 2], F32)

            with tc.tile_pool(name="kvd", bufs=1) as kvd:
                q_t = kvd.tile([128, HPC, Q], BF16)     # [d, h, q] post-norm
                k_t = kvd.tile([128, KVP], BF16)        # [d, kv]
                k2 = kvd.tile([128, KVP], BF16)         # k_t squared
                v_kv = kvd.tile([128, NKC, D], BF16)    # [kv%128, chunk, d]
                kscale = kvd.tile([128, NKC], F32)      # exp scale per kv
                kw = kvd.tile([128, KT, D], BF16)
                vw = kvd.tile([128, KT, D], BF16)
                v_d = kvd.tile([128, KVP], BF16)        # V in [d, kv] form
                attn_t = kvd.tile([128, HPC, Q], BF16)  # normalized A.V

                qn_outer = tc.alloc_tile_pool(name="qn", bufs=1)
                q_f = qn_outer.tile([128, HPC, Q], F32R)  # pre-norm q
                q2 = qn_outer.tile([128, HPC * Q], BF16)

                # ---------------- phase 1: q projection ---------------
                with (
                    tc.tile_pool(name="p1in", bufs=3) as p1in,
                    tc.tile_pool(name="p1ps", bufs=1, space="PSUM") as p1ps,
                ):
                    ps_q = p1ps.tile([128, HPC, Q], F32)  # all 8 banks
                    for s in range(NSLAB1):
                        hts = p1in.tile([128, SLAB_K1, Q], BF16, tag="ht")
                        nc.gpsimd.dma_start(
                            hts[:].rearrange("p k q -> p (k q)"),
                            hid_r[s * HID_SLAB:(s + 1) * HID_SLAB]
                            .rearrange("(p x) -> p x", p=128),
                        )
                        qws = p1in.tile([128, SLAB_K1, HPC * D], BF16,
                                        tag="qw")
                        nc.scalar.dma_start(
                            qws[:],
                            qw_r[s * QW_SLAB:(s + 1) * QW_SLAB]
                            .rearrange("(p k m) -> p k m", p=128, k=SLAB_K1),
                        )
                        for k8 in range(SLAB_K1):
                            k = s * SLAB_K1 + k8
                            for m in range(HPC):
                                for nh in range(2):
                                    nc.tensor.matmul(
                                        ps_q[:, m, nh * 512:(nh + 1) * 512],
                                        lhsT=qws[:, k8, m * 128:(m + 1) * 128],
                                        rhs=hts[:, k8, nh * 512:(nh + 1) * 512],
                                        start=(k == 0), stop=(k == KT - 1),
                                    )
                    nc.vector.tensor_copy(q_f[:], ps_q[:])

                # constants ride the gpsimd queue once phase 1 is underway
                nc.gpsimd.dma_start(onesf[:], ones_f[:])
                nc.gpsimd.dma_start(onesb[:], ones_b[:])
                nc.gpsimd.dma_start(qnw_t[:], qnw[:])
                nc.gpsimd.dma_start(pbias[:], pbias_in[:])
                nc.gpsimd.memset(eps_q[:], EPS)
                nc.gpsimd.memset(eps_k[:], 128.0 * EPS)
                # kv weight loads ride the gpsimd queue behind the consts
                nc.gpsimd.dma_start(kw[:], kw_r[:])
                nc.gpsimd.dma_start(vw[:], vw_r[:])

                qt_f = q_f[:].rearrange("p h q -> p (h q)")
                nc.vector.tensor_mul(q2[:], qt_f, qt_f)

                # ------------- phase 2: k/v projection ----------------
                # (q rmsnorm and per-group exp scales interleaved)
                with (
                    tc.tile_pool(name="fin", bufs=11) as fin,
                    tc.tile_pool(name="fsq", bufs=2) as fsq,
                    tc.tile_pool(name="qn2", bufs=1) as qn,
                    tc.tile_pool(name="fpkv", bufs=1, space="PSUM") as fpkv,
                    tc.tile_pool(name="fpk2", bufs=2, space="PSUM") as fpk2,
                    tc.tile_pool(name="fsqps", bufs=1, space="PSUM") as fsqps,
                    tc.tile_pool(name="qnps", bufs=1, space="PSUM") as qnps,
                ):
                    def kss_group(g):
                        # sumsq -> 1/sqrt for group g's chunks (k2 is ready
                        # well before this is emitted)
                        kv0, w = GROUPS[g]
                        nsub = w // 128
                        kss = fsqps.tile([128, 16], F32, tag="kss",
                                         name="kss")
                        for j in range(nsub):
                            c = g * 8 + j
                            nc.tensor.matmul(
                                kss[:, 2 * j:2 * j + 2],
                                lhsT=k2[:, c * 128:(c + 1) * 128],
                                rhs=onesb[:, 0:2],
                            )
                        ksq = fsq.tile([128, 8], F32, tag="ksq", name="ksq")
                        nc.scalar.activation(
                            ksq[:, :nsub], kss[:, 0:2 * nsub:2],
                            mybir.ActivationFunctionType.Sqrt,
                            bias=eps_k[:], scale=1.0,
                        )
                        nc.vector.reciprocal(
                            kscale[:, g * 8:g * 8 + nsub], ksq[:, :nsub]
                        )

                    def qnorm_slice(i):
                        sl = slice(i * 512, (i + 1) * 512)
                        sb = qnps.tile([128, 512], F32, tag="sb", name="sb")
                        nc.tensor.matmul(sb[:], lhsT=onesb[:], rhs=q2[:, sl])
                        qsb = qn.tile([128, 512], F32, tag="qsb", name="qsb")
                        nc.scalar.activation(
                            qsb[:], sb[:],
                            mybir.ActivationFunctionType.Sqrt,
                            bias=eps_q[:], scale=1.0 / 128,
                        )
                        qrec = qn.tile([128, 512], F32, tag="qrec",
                                       name="qrec")
                        nc.vector.reciprocal_approx_fast(qrec[:], qsb[:])
                        nc.vector.tensor_mul(qt_f[:, sl], qt_f[:, sl],
                                             qrec[:])

                    PROC = [6, 0, 1, 2, 3, 4, 5]
                    for gi, g in enumerate(PROC):
                        kv0, w = GROUPS[g]
                        nh = (w + 511) // 512
                        nsub = w // 128
                        ps_k = fpk2.tile([128, 1024], F32, tag="pk",
                                         name="ps_k")
                        ps_v = fpkv.tile([128, 1024], F32, tag="pv",
                                         name="ps_v")
                        for s in range(NSLAB):
                            if s == 4 and gi >= 2:
                                kss_group(PROC[gi - 2])
                            off = CRS_OFF[g] + s * CRS_SLAB[g]
                            cts = fin.tile([128, SLAB_K, 1024], BF16,
                                           tag="ct")
                            eng = nc.sync if (gi * NSLAB + s) % 2 == 0 \
                                else nc.scalar
                            if w == 1024:
                                eng.dma_start(
                                    cts[:].rearrange("p k c -> p (k c)"),
                                    crs_r[off:off + CRS_SLAB[g]]
                                    .rearrange("(p x) -> p x", p=128),
                                )
                            else:
                                eng.dma_start(
                                    cts[:, :, :w],
                                    crs_r[off:off + CRS_SLAB[g]]
                                    .rearrange("(p k c) -> p k c",
                                               p=128, k=SLAB_K),
                                )
                            for k8 in range(SLAB_K):
                                k = s * SLAB_K + k8
                                for i in range(nh):
                                    cw = min(512, w - i * 512)
                                    nc.tensor.matmul(
                                        ps_k[:, i * 512:i * 512 + cw],
                                        lhsT=kw[:, k, :],
                                        rhs=cts[:, k8, i * 512:i * 512 + cw],
                                        start=(k == 0), stop=(k == KT - 1),
                                    )
                            for k8 in range(SLAB_K):
                                k = s * SLAB_K + k8
                                for i in range(nh):
                                    cw = min(512, w - i * 512)
                                    nc.tensor.matmul(
                                        ps_v[:, i * 512:i * 512 + cw],
                                        lhsT=vw[:, k, :],
                                        rhs=cts[:, k8, i * 512:i * 512 + cw],
                                        start=(k == 0), stop=(k == KT - 1),
                                    )
                        # evacuate K and V (bf16)
                        nc.vector.tensor_copy(
                            k_t[:, kv0:kv0 + w], ps_k[:, :w]
                        )
                        nc.vector.tensor_mul(
                            k2[:, kv0:kv0 + w], k_t[:, kv0:kv0 + w],
                            k_t[:, kv0:kv0 + w],
                        )
                        nc.vector.tensor_copy(
                            v_d[:, kv0:kv0 + w], ps_v[:, :w]
                        )
                        if 1 <= g <= 4:
                            qnorm_slice(2 * (g - 1))
                            qnorm_slice(2 * (g - 1) + 1)
                        if g == 5:
                            # q_norm_w * k_norm_w folded on host into qnw
                            nc.scalar.mul(
                                q_t[:].rearrange("p h q -> p (h q)"),
                                qt_f, qnw_t[:],
                            )
                    kss_group(PROC[-2])
                    kss_group(PROC[-1])
                    # transpose V chunks via the DMA xbar now that the
                    # cross stream no longer competes for the DMA engines
                    for c in range(NKC):
                        nc.sync.dma_start_transpose(
                            v_kv[:, c, :], v_d[:, c * 128:(c + 1) * 128]
                        )
                    # prefetch the Exp table during the phase boundary
                    nc.scalar.activation(
                        scr[0:1, 0:1], eps_q[0:1, :],
                        mybir.ActivationFunctionType.Exp,
                    )
                qn_outer.release()

                # ------- phase 3: attention sweep per head ------------
                with (
                    tc.tile_pool(name="fat", bufs=8) as fat,
                    tc.tile_pool(name="frr", bufs=4) as frr,
                    tc.tile_pool(name="fra", bufs=2) as fra,
                    tc.tile_pool(name="fpss", bufs=2, space="PSUM") as fpss,
                    tc.tile_pool(name="fpo", bufs=1, space="PSUM") as fpo,
                    tc.tile_pool(name="fpr", bufs=1, space="PSUM") as fpr,
                ):
                    for h in range(HPC):
                        ps_o = [fpo.tile([128, 512], F32, tag=f"po{qh}",
                                         name="ps_o") for qh in range(2)]
                        ps_r0 = fpr.tile([128, 512], F32, tag="pr0",
                                         name="ps_r0")
                        racc = fra.tile([128, 512], F32R, tag="racc",
                                        name="racc")
                        # software pipeline: scores one chunk ahead
                        pss = [None] * NKC
                        pss[0] = fpss.tile([128, 1024], F32, tag="pss",
                                           name="ps_s")
                        for qh in range(2):
                            nc.tensor.matmul(
                                pss[0][:, qh * 512:(qh + 1) * 512],
                                lhsT=k_t[:, 0:128],
                                rhs=q_t[:, h, qh * 512:(qh + 1) * 512],
                            )
                        for c in range(NKC):
                            a_t = fat.tile([128, 1024], BF16, tag="at",
                                           name="a_t")
                            nc.scalar.activation(
                                a_t[:], pss[c][:],
                                mybir.ActivationFunctionType.Exp,
                                scale=kscale[:, c:c + 1],
                                bias=(pbias[:] if c == NKC - 1 else 0.0),
                            )
                            if c + 1 < NKC:
                                pss[c + 1] = fpss.tile([128, 1024], F32,
                                                       tag="pss", name="ps_s")
                                for qh in range(2):
                                    nc.tensor.matmul(
                                        pss[c + 1][:, qh * 512:(qh + 1) * 512],
                                        lhsT=k_t[:, (c + 1) * 128:
                                                 (c + 2) * 128],
                                        rhs=q_t[:, h, qh * 512:(qh + 1) * 512],
                                    )
                            for qh in range(2):
                                nc.tensor.matmul(
                                    ps_o[qh][:], lhsT=v_kv[:, c, :],
                                    rhs=a_t[:, qh * 512:(qh + 1) * 512],
                                    start=(c == 0), stop=(c == NKC - 1),
                                )
                            # rowsum: q-half 0 on PE, q-half 1 on Vector
                            nc.tensor.matmul(
                                ps_r0[:], lhsT=onesb[:], rhs=a_t[:, 0:512],
                                start=(c == 0), stop=(c == NKC - 1),
                            )
                            if c == 0:
                                nc.vector.tensor_copy(
                                    racc[:], a_t[:, 512:1024]
                                )
                            else:
                                nc.vector.tensor_add(
                                    racc[:], racc[:], a_t[:, 512:1024]
                                )
                        # normalize: attn = (A.V) / rowsum; overlap the
                        # reciprocal chain with the r1 rowsum matmul
                        rr0 = frr.tile([128, 512], F32, tag="rr0", name="rr0")
                        nc.vector.reciprocal_approx_fast(rr0[:], ps_r0[:])
                        ps_r1 = fpr.tile([128, 512], F32, tag="pr1",
                                         name="ps_r1")
                        nc.tensor.matmul(ps_r1[:], lhsT=onesf[:], rhs=racc[:])
                        nc.vector.tensor_mul(
                            attn_t[:, h, 0:512], ps_o[0][:], rr0[:]
                        )
                        rr1 = frr.tile([128, 512], F32, tag="rr1", name="rr1")
                        nc.vector.reciprocal_approx_fast(rr1[:], ps_r1[:])
                        nc.vector.tensor_mul(
                            attn_t[:, h, 512:1024], ps_o[1][:], rr1[:]
                        )

                # ------------- phase 4: o projection ------------------
                with (
                    tc.tile_pool(name="p4w", bufs=4) as p4w,
                    tc.tile_pool(name="p4o", bufs=4) as p4o,
                    tc.tile_pool(name="p4ps", bufs=6, space="PSUM") as p4ps,
                ):
                    for oc in range(H // 512):
                        owc = p4w.tile([128, HPC, 512], BF16, tag="owc")
                        nc.scalar.dma_start(
                            owc[:], ow_r[:, :, oc * 512:(oc + 1) * 512]
                        )
                        for half in range(2):
                            ots = p4o.tile([128, 4, 512], F32, tag="ot")
                            for q4 in range(4):
                                qc = half * 4 + q4
                                ps = p4ps.tile([128, 512], F32, tag="ps4")
                                for h in range(HPC):
                                    nc.tensor.matmul(
                                        ps[:],
                                        lhsT=attn_t[:, h,
                                                    qc * 128:(qc + 1) * 128],
                                        rhs=owc[:, h, :],
                                        start=(h == 0), stop=(h == HPC - 1),
                                    )
                                nc.vector.tensor_copy(ots[:, q4, :], ps[:])
                            off4 = oc * OUT_SLAB + half * (OUT_SLAB // 2)
                            nc.sync.dma_start(
                                out[off4:off4 + OUT_SLAB // 2]
                                .rearrange("(p q o) -> p q o", p=128, q=4),
                                ots[:],
                            )
    nc.finalize()
    return nc


_NC_CACHE = None


def _get_nc():
    global _NC_CACHE
    if _NC_CACHE is None:
        _NC_CACHE = build_nc()
    return _NC_CACHE


def unblock_out(arr):
    """[8 oc, 2 half, 128 p, 4 qc, 512 o] blocked -> [Q, H]."""
    return (arr.reshape(8, 2, 128, 4, 512).transpose(1, 3, 2, 0, 4)
            .reshape(Q, H))


def make_in_maps(inputs):
    hidden = np.asarray(inputs["hidden_states"], np.float32)
    cross = np.asarray(inputs["cross_attention_states"], np.float32)
    qw = np.asarray(inputs["q_proj_w"], np.float32)
    kw = np.asarray(inputs["k_proj_w"], np.float32)
    vw = np.asarray(inputs["v_proj_w"], np.float32)
    ow = np.asarray(inputs["o_proj_w"], np.float32)
    qnw = np.asarray(inputs["q_norm_w"], np.float32).reshape(D, 1)
    knw = np.asarray(inputs["k_norm_w"], np.float32).reshape(D, 1)

    hid_t = np.ascontiguousarray(hidden[0].T).astype(NPBF16)   # [H, Q]
    hid_r = np.ascontiguousarray(
        hid_t.reshape(NSLAB1, SLAB_K1, 128, Q).transpose(0, 2, 1, 3)
    ).ravel()
    crs_t = np.zeros((H, KVP), NPBF16)                         # [H, KVP]
    crs_t[:, :KV] = np.ascontiguousarray(cross[0].T)
    crs4 = crs_t.reshape(NSLAB, SLAB_K, 128, KVP)              # [s, k, p, c]
    crs_parts = []
    for (kv0, w) in GROUPS:
        blk = crs4[:, :, :, kv0:kv0 + w]                       # [s, k, p, w]
        crs_parts.append(
            np.ascontiguousarray(blk.transpose(0, 2, 1, 3)).ravel()
        )
    crs_r = np.concatenate(crs_parts)
    qwb = qw.astype(NPBF16)
    kwb = kw.astype(NPBF16)
    vwb = vw.astype(NPBF16)
    owb = ow.astype(NPBF16)
    ones_f = np.ones((128, 128), np.float32)
    ones_b = np.ones((128, 128), NPBF16)
    pbias = np.zeros((128, 1), np.float32)
    pbias[LAST_VALID:] = -40.0
    in_maps = []
    for c in range(8):
        qwc = np.ascontiguousarray(qwb[512 * c:512 * (c + 1), :].T)  # [H,512]
        qw_rb = np.ascontiguousarray(
            qwc.reshape(NSLAB1, SLAB_K1, 128, HPC * D).transpose(0, 2, 1, 3)
        ).ravel()
        kw_r = np.ascontiguousarray(
            kwb[128 * c:128 * (c + 1), :].reshape(128, KT, 128)
            .transpose(2, 1, 0)
        )
        vw_r = np.ascontiguousarray(
            vwb[128 * c:128 * (c + 1), :].reshape(128, KT, 128)
            .transpose(2, 1, 0)
        )
        ow_r = np.ascontiguousarray(
            owb[:, 512 * c:512 * (c + 1)].reshape(H, HPC, 128)
            .transpose(2, 1, 0)
        )
        in_maps.append({
            "hid_r": hid_r,
            "crs_r": crs_r,
            "qw_r": qw_rb,
            "kw_r": kw_r,
            "vw_r": vw_r,
            "ow_r": ow_r,
            "ones_f": ones_f,
            "ones_b": ones_b,
            "qnw": qnw * knw,
            "pbias": pbias,
        })
    return in_maps


def kernel(**inputs) -> np.ndarray:
    nc = _get_nc()
    res = run_bass_kernel_spmd(nc, make_in_maps(inputs), core_ids=list(range(8)))
    acc = np.zeros(Q * H, np.float64)
    for c in range(8):
        acc += res.results[c]["out"]
    return unblock_out(acc.astype(np.float32)).reshape(1, Q, H)
